# revision 41
# baseline (speedup 1.0000x reference)
"""Trainium2 Bass kernel for AttentionConv2d (self-attention over 64x64 pixels).

Reference math (per image b):
    xf = x.reshape(C, N)                      # C=256, N=4096
    q  = Wq @ xf + bq                         # [32, N]
    k  = Wk @ xf + bk                         # [32, N]
    v  = Wv @ xf + bv                         # [256, N]
    corr[i, j] = sum_c q[c, i] * k[c, j]      # [N, N]
    beta = softmax(corr, axis=0)              # over i, per column j
    att[c, j] = gamma * sum_i v[c, i] * beta[i, j]
    out = att.reshape(C, H, W) + x

Sharding: data-parallel over batch, one image per NeuronCore (8 cores).

Measured ~188us vs the 198.5us predecessor.  The kernel is PE-bound
(Tensor busy ~164us: PV 512xN=512 bf16 matmuls at the 216ns streaming
floor + corr quads + projections); the Scalar-engine exp stream (128
ACTIVATEs x 1.11us = 142us) runs eagerly ahead and its stalls are
absorbed.  Wall time = PE-stream span + output tail, so the design
minimizes PE idle and the post-stream tail:
  - input DMA: weights on the scalar queue, x on the sync queue in 16
    ordered half-chunk transfers (x is host-packed [128, 2, 4096]); each
    transfer stripes over all 16 DMA engines, so chunk c completes ~1.8us
    after c-1 and the first projection starts ~13us in.
  - PE warmup: 15 N=512 dummy matmuls bridge until chunk 0 lands, plus
    keep-warm fillers between the first three (DMA-paced) chunks -- the
    HAM clock gate releases at ~11us (1.2 -> 2.4 GHz) and never re-arms
    (a >3.4us PE-idle window would halve the clock).
  - q/k projection PSUMs live in the att-accumulator banks (idle until the
    first PV) so the eps pool only rotates corr quads + v^T tiles.
  - corr matmuls are 4x row-tiled (tile_position=(32r,0), K=32 strips); q/k
    are produced 4x-replicated by widening the projection weights host-side.
    (Narrower j-blocks would halve the drain further, but two row-strips
    writing one PSUM bank hang real HW -- blocks stay 512 wide.)
  - E is stored in per-quad bf16 tiles ([128,2048], 17-deep pool): fine WAR
    granularity so the eager exp stream never waits on block recycling.
  - flat software pipeline over quads: PV starts LAG=8 quads behind the
    exps and claws back GAIN=6 quads over the run (the PE's ~2us/block
    slack vs ACT absorbs it), so only ~2 PV quads drain after the last
    exp instead of 8 (~13us -> ~5us tail).  The last block's h1 PV trails
    h0 by one quad so att0's output chain overlaps the final PV matmuls.
  - softmax denominator: per-quad bf16 adds on DVE, emitted DLAG=3 quads
    behind the exps (a lag-1 pair-add waiting on its exp would head-block
    the strict-FIFO DVE queue and delay the att-releasing muls behind it).
    Partition-reduce + broadcast on the idle GpSimd engine for blocks 1-4;
    blocks 0 and 5-7 use a ones-matmul broadcast instead (block 0: the
    DVE reciprocal waiting 3.8us on gpsimd head-blocks the ramp-congested
    DVE queue; blocks 5-7: the shrunken PV lag is tighter than the gpsimd
    latency).  The reciprocal runs in out_tail, by which point the reduce
    has long finished.
  - out_tail: both att-releasing muls first (frees the single-buffered att
    banks for the next block's PV), then residual adds and output DMAs on
    the sync queue (last block: h1 on the then-idle scalar queue).
gamma is folded into Wv host-side; gamma*bv is added at the end (softmax
weights sum to 1, so the v-bias is a per-channel constant).
"""

import sys

sys.path.insert(0, "/opt/trn_rl_repo")

from contextlib import ExitStack

import numpy as np

C = 256
CR = 32
N = 4096
CH = 128          # channel half (partition dim)
IT = 128          # i-tile height (partition dim of E tiles)
NC = 8            # x column chunks (512 wide)
NWIDE = 8         # 512-wide j-blocks
NNARROW = 0       # 256-wide j-blocks (disabled: narrow corr matmuls --
                  # two row-strips into one PSUM bank -- hang real HW)
LAG = 8           # initial quads of PV lag behind corr/exp
GAIN = 6          # quads of PV lag clawed back linearly after the ramp:
                  # only ~2 PV quads drain after the last exp instead of 8
DLAG = 3          # quads of denominator-accumulation lag behind exp: the
                  # pair-add for quad q waits on exp(q); with lag 1 that
                  # wait sits at the head of the strict-FIFO DVE queue and
                  # delays att-release muls behind it
NQ = 8 * (NWIDE + NNARROW)   # 64 quads total

# (j0, width) per block; quad qi -> block qi//8, sub-quad g = qi%8,
# i-tiles 4g..4g+3
BLOCKS = [(512 * b, 512) for b in range(NWIDE)] + [
    (512 * NWIDE + 256 * b, 256) for b in range(NNARROW)
]


def _build_program():
    import concourse.bass as bass
    import concourse.mybir as mybir
    from concourse import bacc, bass_isa, tile

    f32 = mybir.dt.float32
    f32r = mybir.dt.float32r
    bf16 = mybir.dt.bfloat16
    EXP = mybir.ActivationFunctionType.Exp
    ADD = mybir.AluOpType.add
    ts = bass.ts

    nc = bacc.Bacc()
    # x host-packed as [128, 2, 4096]: [p, h, col] = xf[h*128+p, col] so one
    # 3D-AP transfer moves a full [256, 512] chunk into xf[:, c*1024:+1024]
    x_d = nc.declare_dram_parameter("x", [CH, 2, N], f32r, isOutput=False)
    # wpack host-packed as [128, 1024]: [p, h*512 + {0:128 wq4, 128:256 wk4,
    # 256:512 wvt}] (one transfer)
    wpack_d = nc.declare_dram_parameter("wpack", [CH, 1024], f32r, isOutput=False)
    bpack_d = nc.declare_dram_parameter("bpack", [128, 4], f32, isOutput=False)
    out_d = nc.declare_dram_parameter("out", [C, N], f32, isOutput=True)

    with TileCtx(tile, nc) as (tc, ctx):
        const = ctx.enter_context(tc.tile_pool(name="const", bufs=1))
        vtp = ctx.enter_context(tc.tile_pool(name="vtp", bufs=1))
        ebw = ctx.enter_context(tc.tile_pool(name="ebw", bufs=17))   # wide E quads
        pairp = ctx.enter_context(tc.tile_pool(name="pairp", bufs=2))
        accp = ctx.enter_context(tc.tile_pool(name="accp", bufs=2))
        rbp = ctx.enter_context(tc.tile_pool(name="rbp", bufs=2))
        scrp = ctx.enter_context(tc.tile_pool(name="scrp", bufs=2))
        outp = ctx.enter_context(tc.tile_pool(name="outp", bufs=4))
        # PSUM: eps 3x[128,1024] = 6 banks (corr quads, v^T) + 2x[128,512]
        # att accumulators = 8 banks.  att banks double as the q/k
        # projection PSUMs during the head (PV starts long after).
        eps_p = ctx.enter_context(tc.tile_pool(name="eps_p", bufs=3, space="PSUM"))
        att0_p = ctx.enter_context(tc.tile_pool(name="att0_p", bufs=1, space="PSUM"))
        att1_p = ctx.enter_context(tc.tile_pool(name="att1_p", bufs=1, space="PSUM"))

        # ---- resident inputs --------------------------------------------
        # weights on the scalar queue (parallel with x), x chunks in order
        # on the sync queue so chunk c completes ~1.7us after chunk c-1
        wtile = const.tile([CH, 1024], f32r, name="wtile")
        nc.scalar.dma_start(out=wtile[:], in_=wpack_d[:, :])
        bpack = const.tile([128, 4], f32, name="bpack")
        nc.scalar.dma_start(out=bpack[:], in_=bpack_d[:, :])
        xf = const.tile([CH, NC * 1024], f32r, name="xf")
        for c in range(NC):
            for h in range(2):
                nc.sync.dma_start(
                    out=xf[:, c * 1024 + h * 512:c * 1024 + (h + 1) * 512],
                    in_=x_d[:, h, ts(c, 512)],
                )

        def xcol(c, h, off=0):
            return c * 1024 + h * 512 + off

        wq4t = [wtile[:, h * 512 + 0:h * 512 + 128] for h in range(2)]
        wk4t = [wtile[:, h * 512 + 128:h * 512 + 256] for h in range(2)]
        wvt = [wtile[:, h * 512 + 256:h * 512 + 512] for h in range(2)]
        bq4_t = bpack[:, 0:1]
        bk4_t = bpack[:, 1:2]
        gbv = [bpack[:, 2 + h:3 + h] for h in range(2)]

        ones512 = const.tile([128, 512], bf16, name="ones512")
        nc.vector.memset(ones512[:], 1.0)
        ones_b = const.tile([128, 128], bf16, name="ones_b")
        nc.vector.memset(ones_b[:], 1.0)
        # PE warmup: ~5us of cold N=512 dummy matmuls bridging the gap
        # until chunk 0 lands, so the HAM clock gate sees one fully-busy
        # 3.4us window and releases (1.2 -> 2.4 GHz) before the first
        # projection; a PE idle hole here would restart the wait.
        warm = eps_p.tile([128, 1024], f32, tag="eps", name="eps")
        for _ in range(15):
            nc.tensor.matmul(
                warm[:, 0:512], lhsT=ones_b[:], rhs=ones512[:], start=True, stop=True
            )

        q4 = const.tile([128, N], bf16, name="q4")
        k4 = const.tile([128, N], bf16, name="k4")
        vt = []
        equads = {}

        def corr_exp_quad(qi):
            """4x row-tiled corr matmuls + exp(s) for quad qi."""
            bi, g = divmod(qi, 8)
            j0, w = BLOCKS[bi]
            if w == 512:
                epsA = eps_p.tile([128, 1024], f32, tag="eps", name="eps")
                epsB = eps_p.tile([128, 1024], f32, tag="eps", name="eps")
                for r in range(4):
                    i = 4 * g + r
                    dst = epsA if r < 2 else epsB
                    nc.tensor.matmul(
                        dst[:, ts(r % 2, 512)],
                        lhsT=q4[32 * r:32 * (r + 1), ts(i, IT)],
                        rhs=k4[32 * r:32 * (r + 1), j0:j0 + 512],
                        start=True,
                        stop=True,
                        tile_position=(32 * r, 0),
                    )
                eq = ebw.tile([IT, 2048], bf16, tag="eq", name="eq")
                nc.scalar.activation(eq[:, 0:1024], epsA[:], EXP)
                nc.scalar.activation(eq[:, 1024:2048], epsB[:], EXP)
            else:
                # narrow blocks are disabled (NNARROW=0): 256-wide corr
                # strips put two concurrent row-strip matmuls into one PSUM
                # bank, which hangs real HW
                raise NotImplementedError("narrow j-blocks hang HW")
            equads[qi] = eq

        # ---- head: x chunks -> projections, v^T, block-0 corr/exp --------
        for c in range(NC):
            csl = ts(c, 512)
            for (dst, wt, bias, pool) in (
                (q4, wq4t, bq4_t, att0_p),
                (k4, wk4t, bk4_t, att1_p),
            ):
                ps = pool.tile([CH, 512], f32, tag="ps", name="ps")
                for h in range(2):
                    nc.tensor.matmul(
                        ps[:],
                        lhsT=wt[h],
                        rhs=xf[:, xcol(c, h):xcol(c, h) + 512],
                        start=(h == 0),
                        stop=(h == 1),
                    )
                nc.vector.tensor_scalar_add(dst[:, csl], ps[:], bias)
            corr_exp_quad(c)
            psv = eps_p.tile([128, 1024], f32, tag="eps", name="eps")
            for t4 in range(4):
                for h in range(2):
                    nc.tensor.matmul(
                        psv[:, ts(t4, C)],
                        lhsT=xf[:, xcol(c, h, t4 * 128):xcol(c, h, t4 * 128) + 128],
                        rhs=wvt[h],
                        start=(h == 0),
                        stop=(h == 1),
                    )
            vtile = vtp.tile([128, 1024], bf16, name=f"vt{c}")
            # force DVE: nc.any let the scheduler put half of these on the
            # Scalar engine, delaying the exp stream
            nc.vector.tensor_copy(vtile[:], psv[:])
            vt.append(vtile)
            if c < 3:
                # keep-warm fillers between the DMA-paced early chunks: the
                # PE idle while waiting for the next x chunk would re-arm
                # the HAM throttle and halve the clock for the next chunk
                for _ in range(2):
                    nc.tensor.matmul(
                        warm[:, 0:512], lhsT=ones_b[:], rhs=ones512[:],
                        start=True, stop=True,
                    )

        # ---- denominator / PV / output helpers ----------------------------
        atts = {}
        rbs = {}
        accs = {}
        srcs = {}

        def den_step(qi):
            """Accumulate exp sums for quad qi (lags exp by DLAG quads)."""
            bi, g = divmod(qi, 8)
            j0, w = BLOCKS[bi]
            eq = equads[qi]
            if w == 512:
                pair = pairp.tile([128, 1024], bf16, tag="pair", name="pair")
                nc.vector.tensor_add(pair[:], eq[:, 0:1024], eq[:, 1024:2048])
                src = pair[:]
            else:
                src = eq[:]
            if g == 0:
                accs[bi] = accp.tile([128, 1024], bf16, tag="acc", name="acc")
                nc.vector.tensor_copy(accs[bi][:], src)
            else:
                nc.vector.tensor_add(accs[bi][:], accs[bi][:], src)
            if g == 7:
                den_tail(bi)

        def den_tail(bi):
            j0, w = BLOCKS[bi]
            acc = accs[bi]
            nc.vector.tensor_add(acc[:, 0:512], acc[:, 0:512], acc[:, 512:1024])
            if w == 256:
                nc.vector.tensor_add(acc[:, 0:256], acc[:, 0:256], acc[:, 256:512])
            s_part = acc[:, 0:w]
            if 1 <= bi < 5:
                # partition-reduce + broadcast on the (otherwise idle)
                # GpSimd engine: zero PE cost.  The reciprocal is deferred
                # to out_tail so its wait on the 3.8us gpsimd op never
                # head-blocks the strict-FIFO DVE queue.
                s_bc = scrp.tile([CH, w], f32, tag="s_bc", name="s_bc")
                nc.gpsimd.partition_all_reduce(
                    s_bc[:], s_part, channels=CH, reduce_op=bass_isa.ReduceOp.add
                )
                srcs[bi] = s_bc[:]
            else:
                # late blocks: ones-matmul broadcast -- by now the PV lag
                # has shrunk below the 3.8us gpsimd latency, and a waiting
                # reciprocal would head-block the DVE queue
                smm = eps_p.tile([128, 1024], f32, tag="eps", name="eps")
                nc.tensor.matmul(
                    smm[:, 0:w], lhsT=ones_b[:], rhs=s_part, start=True, stop=True
                )
                srcs[bi] = smm[:, 0:w]

        def pv_half(qi, h):
            bi, g = divmod(qi, 8)
            j0, w = BLOCKS[bi]
            eq = equads[qi]
            for t4 in range(4):
                i = 4 * g + t4
                nc.tensor.matmul(
                    atts[bi][h][:],
                    lhsT=vt[g][:, t4 * C + h * CH:t4 * C + (h + 1) * CH],
                    rhs=eq[:, ts(t4, w)],
                    start=(i == 0),
                    stop=(i == 31),
                )

        def pv_quad(qi):
            """PV accumulation matmuls for quad qi (lags exp by LAG quads)."""
            bi, g = divmod(qi, 8)
            if g == 0:
                j0, w = BLOCKS[bi]
                atts[bi] = (
                    att0_p.tile([CH, w], f32, tag="ps", name="ps"),
                    att1_p.tile([CH, w], f32, tag="ps", name="ps"),
                )
            if bi == len(BLOCKS) - 1:
                # last block: h1 trails h0 by one quad, so att0 completes a
                # quad early and its output chain overlaps the final PV
                pv_half(qi, 0)
                if g > 0:
                    pv_half(qi - 1, 1)
                if g == 7:
                    pv_half(qi, 1)
            else:
                for t4 in range(4):
                    for h in range(2):
                        i = 4 * (qi % 8) + t4
                        eq = equads[qi]
                        j0, w = BLOCKS[bi]
                        nc.tensor.matmul(
                            atts[bi][h][:],
                            lhsT=vt[qi % 8][:, t4 * C + h * CH:t4 * C + (h + 1) * CH],
                            rhs=eq[:, ts(t4, w)],
                            start=(i == 0),
                            stop=(i == 31),
                        )

        def out_tail(bi):
            j0, w = BLOCKS[bi]
            c, off = divmod(j0, 512)  # xf chunk/offset for the residual
            last = bi == len(BLOCKS) - 1
            # reciprocal here (not in den_tail): by now the gpsimd reduce
            # has long finished, so this never blocks the DVE queue
            rb = rbp.tile([CH, w], f32, tag="rb", name="rb")
            rscr = scrp.tile([CH, w], f32, tag="rscr", name="rscr")
            nc.vector.reciprocal_approx_accurate(out=rb[:], in_=srcs[bi], scratch=rscr[:])
            rbs[bi] = rb

            def _mul(h):
                o = outp.tile([CH, w], f32, tag="o", name="o")
                nc.vector.tensor_mul(o[:], atts[bi][h][:], rbs[bi][:])
                return o

            def _sttdma(h, o):
                xsl = xf[:, xcol(c, h, off):xcol(c, h, off) + w]
                nc.vector.scalar_tensor_tensor(
                    out=o[:], in0=o[:], scalar=gbv[h], in1=xsl.bitcast(f32),
                    op0=ADD, op1=ADD,
                )
                # last block: h1 issue on the (now idle) scalar queue so the
                # two output transfers pipeline
                eng = nc.scalar if (last and h == 1) else nc.sync
                eng.dma_start(out=out_d[h * CH:(h + 1) * CH, j0:j0 + w], in_=o[:])

            if last:
                # drain order: get h0's output DMA moving while h1 wraps up
                o0 = _mul(0)
                _sttdma(0, o0)
                o1 = _mul(1)
                _sttdma(1, o1)
            else:
                # both att-releasing muls first (frees both att banks ASAP
                # for the next block's PV), then the residual adds and DMAs
                os = [_mul(0), _mul(1)]
                _sttdma(0, os[0])
                _sttdma(1, os[1])

        # ---- main flat pipeline over quads --------------------------------
        # exp side leads; PV trails with a lag that shrinks from LAG to
        # LAG-GAIN over the run (the PE's per-block slack absorbs the extra
        # quads), so the post-exp PV drain is ~2 quads, not 8.  Denominator
        # accumulation lags by DLAG and is emitted after out_tail's muls so
        # those win the strict-FIFO DVE queue.
        pv_cursor = 0

        den_cursor = 0
        ot_count = [0]   # out_tails emitted so far

        def pv_advance(target):
            nonlocal pv_cursor
            while pv_cursor < min(target, NQ):
                pv_quad(pv_cursor)
                if pv_cursor % 8 == 7:
                    out_tail(pv_cursor // 8)
                    ot_count[0] += 1
                pv_cursor += 1

        def den_advance(k):
            nonlocal den_cursor
            lim = min(k - DLAG + 1, NQ)
            while den_cursor < lim:
                den_step(den_cursor)
                den_cursor += 1

        for k in range(DLAG, NQ):
            if 8 <= k:
                corr_exp_quad(k)
            den_advance(k)
            # lag starts shrinking only at k=24: during the DMA-paced ramp
            # an early PV quad would sit in the strict PE FIFO ahead of
            # block-1's corr and stall the exp stream ~7us
            pv_advance(k - LAG + (GAIN * max(0, k - 24)) // (NQ - 24) + 1)
            den_advance(k)  # release dens a fresh out_tail just unlocked
        # drain: pv through quad NQ-2 first so the last block's ones-matmul
        # (which waits on the DVE acc fold) doesn't head-block the PE FIFO
        # during the PV drain
        pv_advance(NQ - 1)
        den_advance(NQ + DLAG)
        pv_advance(NQ)

    nc.finalize()
    return nc


class TileCtx:
    """with TileCtx(tile, nc) as (tc, ctx): ... -- TileContext + ExitStack."""

    def __init__(self, tile_mod, nc):
        self.tc = tile_mod.TileContext(nc)
        self.ctx = ExitStack()

    def __enter__(self):
        self.tc.__enter__()
        self.ctx.__enter__()
        return self.tc, self.ctx

    def __exit__(self, *exc):
        self.ctx.__exit__(*exc)
        return self.tc.__exit__(*exc)


def _run(x, Wq, bq, Wk, bk, Wv, bv, gamma, trace=False, tmpdir=None):
    from concourse.bass_utils import run_bass_kernel_spmd

    B = x.shape[0]
    g = float(np.asarray(gamma).reshape(-1)[0])

    f32 = np.float32
    wq4 = np.tile(np.asarray(Wq, dtype=f32).T, (1, 4))      # [256, 128]
    wk4 = np.tile(np.asarray(Wk, dtype=f32).T, (1, 4))      # [256, 128]
    wvt = (g * np.asarray(Wv, dtype=f32)).T                  # [256, 256]
    wpack = np.concatenate([wq4, wk4, wvt], axis=1)          # [256, 512]
    # -> [128, 1024]: [p, h*512 + col]
    wpack = np.ascontiguousarray(
        wpack.reshape(2, 128, 512).transpose(1, 0, 2).reshape(128, 1024)
    )
    bq4 = np.tile(np.asarray(bq, dtype=f32), 4).reshape(128, 1)
    bk4 = np.tile(np.asarray(bk, dtype=f32), 4).reshape(128, 1)
    gbv = (g * np.asarray(bv, dtype=f32)).reshape(C, 1)
    bpack = np.ascontiguousarray(
        np.concatenate([bq4, bk4, gbv[0:128], gbv[128:256]], axis=1)
    )

    nc = _build_program()

    in_maps = []
    for b in range(B):
        xb = np.asarray(x[b], dtype=f32).reshape(C, N)
        # [128, 2, 4096]: [p, h, col] = xb[h*128+p, col]
        xb = np.ascontiguousarray(xb.reshape(2, 128, N).transpose(1, 0, 2))
        in_maps.append({"x": xb, "wpack": wpack, "bpack": bpack})
    res = run_bass_kernel_spmd(
        nc, in_maps, core_ids=list(range(B)), trace=trace, tmpdir=tmpdir
    )
    out = np.stack([res.results[b]["out"] for b in range(B)], axis=0)
    out = out.reshape(x.shape).astype(np.float32)
    return out, res


def kernel(x, Wq, bq, Wk, bk, Wv, bv, gamma):
    out, _ = _run(x, Wq, bq, Wk, bk, Wv, bv, gamma, trace=False)
    return out
